# revision 39
# baseline (speedup 1.0000x reference)
"""Trainium2 Bass kernel for a linear state-space scan.

Reference computation (per batch row b):
    x_{t+1} = A x_t + B u_t          (x_0 = 0)
    out[t]  = C x_{t+1} + D u_t  =  E x_t + F u_t
with E = C A, F = C B + D.

Shapes: u [32, 4096, 128]; A, B, C, D [128, 128]; out [32, 4096, 128].

Strategy
--------
Data-parallel over batch: 32 rows / 8 cores = 4 rows per core; A/B/C/D-derived
weights replicated. No collectives.

Per core, time is split into N1 chunks of length L (L*N1 = 4096). Because
A = 0.9 * orthogonal, ||A^m|| = 0.9^m exactly, so chunk-start states are a
*short* truncated convolution over previous chunk contributions (J terms,
error ~0.9^(L*J)), making the whole computation chunk-parallel:

  1. Local scan (zero-init per chunk), all chunks in parallel as matmul
     columns:  w_{c,j+1} = A w_{c,j} + B u_{c,j}  -> L sequential matmuls
     over [128, N1*4] column slabs.  The slab is split into 512-column
     chunks forming independent scan chains, interleaved on PE so the
     PSUM->SBUF copy of one chain hides under the matmuls of others.
  2. Chunk-start states: s_c = sum_{m<J} (A^L)^m R_{c-1-m},  R_c = w_{c,L}.
  3. Outputs: out[c,j] = E w_{c,j} + F u_{c,j} + (E A^j) s_c.

Everything on-chip is kept transposed ([d=128 partitions, columns]); the
host does the (cheap) layout permutations during shard/unshard.
"""

import numpy as np

import concourse.tile as tile
from concourse import bacc, mybir
from concourse.bass_utils import run_bass_kernel_spmd

# Problem constants (hardcoded per contract)
B_SZ, T, DIM = 32, 4096, 128
N_CORES = 8
B_PER = B_SZ // N_CORES      # 4 batch rows per core

PS_DT = mybir.dt.float32

_MM_DTS = {"f32r": mybir.dt.float32r, "bf16": mybir.dt.bfloat16,
           "fp16": mybir.dt.float16}

# On-chip matmul dtype. "f32r" (fp32 storage, ~tf32 matmul), "bf16", or
# "fp16" (half storage, 10-bit mantissa -- 8x less quantization error than
# bf16 at the same speed; all values here are << fp16 range).
# NOTE: a bf16 matmul followed by an f32r matmul hangs the PE (weight-path
# hazard, repro.py) -- so 2-byte and f32r modes must not be mixed.
DT_MODE = "fp16"
MM_DT = _MM_DTS[DT_MODE]
Y_HALF = True                # ship y in the 2-byte dtype (halves DMA-out)


def set_dtypes(mode, y_half=None):
    global DT_MODE, MM_DT, Y_HALF
    DT_MODE = mode
    MM_DT = _MM_DTS[mode]
    if y_half is not None:
        Y_HALF = y_half


def _np_dt():
    import ml_dtypes
    return {"f32r": np.float32, "bf16": ml_dtypes.bfloat16,
            "fp16": np.float16}[DT_MODE]

CW = 512                     # matmul column-chunk width (one PSUM bank of f32)
PSY_BUFS = 1                 # 4 psum tags (y/y2 x chunk) -> 4 banks at bufs=1
SCAN_BUFS = 2
UNROLL = 4                   # bodies per hardware-loop iteration
WSPLIT = True                # split weight DMA into phased slices
YDMA_ACT = True              # issue y DMAs on the Act HWDGE queue


def set_chunk(Lnew, Jnew):
    """Set chunk length L (scan steps) and boundary truncation J.

    Double-step layout: the scan advances two steps at a time
    (w_{j+2} = A^2 w_j + AB u_j + B u_{j+1}), so only even-index state
    slabs exist. Weight slices:
      0:A^2  1:AB  2:B  3:E  4:EA  5:EB  6:F
      7..7+L/2-2:      A^j for even j in [2, L-2]
      7+L/2-1 .. +J-1: M_m = (A^L)^m truncated-conv weights
    """
    global L, N1, COLS, J, NW, NCH
    global W_A2, W_AB, W_B, W_E, W_EA, W_EB, W_F, W_P0, W_M0
    L = Lnew
    N1 = T // L
    COLS = N1 * B_PER
    J = Jnew
    W_A2, W_AB, W_B, W_E, W_EA, W_EB, W_F = 0, 1, 2, 3, 4, 5, 6
    W_P0 = 7                 # A^j, even j >= 2: index W_P0 + j//2 - 1
    W_M0 = W_P0 + L // 2 - 1
    NW = W_M0 + J
    NCH = COLS // CW


set_chunk(16, 5)             # 0.9^(16*5) ~ 2.2e-4 truncation, << fp16 noise



MM_SPLIT = 256               # moving-dim split: 256-col matmuls run at full
                             # PE clock on this part, 512-col ones ~1.5x slower


def _mm(nc, out_ap, w_ap, rhs_ap, ncols, start, stop):
    """matmul with the moving dim split into MM_SPLIT-col pieces."""
    lo = 0
    while lo < ncols:
        hi = min(lo + MM_SPLIT, ncols)
        nc.tensor.matmul(out_ap[:, lo:hi], w_ap, rhs_ap[:, lo:hi],
                         start=start and lo == 0,
                         stop=stop and hi == ncols)
        lo = hi


def _build_program(loop_reps=1, variant="full"):
    nc = bacc.Bacc("TRN2", target_bir_lowering=False, debug=False,
                   num_devices=N_CORES)
    u_dt = MM_DT
    y_dt = MM_DT if (Y_HALF and DT_MODE != "f32r") else PS_DT
    ut = nc.dram_tensor("ut", [L, 128, COLS], u_dt, kind="ExternalInput")
    wt = nc.dram_tensor("wt", [128, NW * 128], MM_DT, kind="ExternalInput")
    if variant in ("dma", "dmaint", "dmain", "dmaout"):
        yt_dt = u_dt
    elif variant == "scan":
        yt_dt = MM_DT
    else:
        yt_dt = y_dt
    yt = nc.dram_tensor("yt", [L, 128, COLS], yt_dt, kind="ExternalOutput")

    with tile.TileContext(nc) as tc:
        from contextlib import ExitStack
        ctx = ExitStack()
        with (
            # bufs=2 on cross-iteration tiles: under a hardware loop, a
            # single-buffered tile's WAR (next rep's DMA write vs this rep's
            # phase C read) blocks the SP queue head and serializes reps.
            tc.tile_pool(name="wts", bufs=2) as wpool,
            tc.tile_pool(name="u", bufs=2) as upool,
            tc.tile_pool(name="x", bufs=2) as xpool,
            tc.tile_pool(name="s", bufs=2) as spool,
            tc.tile_pool(name="y", bufs=6) as ypool,
            tc.tile_pool(name="ps", bufs=SCAN_BUFS, space="PSUM") as pspool,
            tc.tile_pool(name="psy", bufs=PSY_BUFS, space="PSUM") as psypool,
            ctx,
        ):
            wt_r = wt.ap().rearrange("p (n d) -> p n d", n=NW)

            def cc(h):  # column-chunk slice
                return slice(h * CW, (h + 1) * CW)

            unroll = 1
            if loop_reps > 1:
                # Unroll inside the hardware loop so double-buffered pools
                # actually alternate slots across consecutive reps.
                assert loop_reps % UNROLL == 0, "loop_reps % UNROLL != 0"
                ctx.enter_context(tc.For_i(0, loop_reps // UNROLL, 1))
                unroll = UNROLL

            for _rep in range(unroll):
                _emit_body(nc, tc, variant, _rep, wpool, upool, xpool, spool,
                           ypool, pspool, psypool, ut, wt_r, yt, u_dt, y_dt,
                           cc)

    nc.compile()
    return nc


def _emit_body(nc, tc, variant, rep, wpool, upool, xpool, spool, ypool,
               pspool, psypool, ut, wt_r, yt, u_dt, y_dt, cc):
    wtile = wpool.tile([128, NW, 128], MM_DT, tag="w", name=f"wtile{rep}")

    def w(i):
        return wtile[:, i, :]

    def wB():
        return w(W_B)

    def wF():
        return w(W_F)

    if True:
        if True:
            # Scan weights (A^2, AB, B) first (tiny DMA) so phase A can start
            # immediately; the remaining weight slices are interleaved into
            # the u stream early enough for phases B/C but without starving
            # phase A.
            wrest = [(W_E, W_P0), (W_P0, W_M0), (W_M0, NW)]
            wload_after = {6: 0, 10: 1, 12: 2}
            if WSPLIT:
                nc.sync.dma_start(wtile[:, :W_E], wt_r[:, :W_E])
            else:
                nc.sync.dma_start(wtile[:], wt_r[:])

            n_loads = 1 if variant == "dmaout" else L
            u_tiles = []
            lag = 4
            for j in range(n_loads):
                u_j = upool.tile([128, COLS], u_dt, tag=f"u{j}",
                                 name=f"u{j}_{rep}")
                nc.sync.dma_start(u_j[:], ut[j])
                u_tiles.append(u_j)
                if WSPLIT and j in wload_after:
                    lo, hi = wrest[wload_after[j]]
                    nc.sync.dma_start(wtile[:, lo:hi], wt_r[:, lo:hi])
                if variant == "dmaint" and j >= lag:
                    nc.sync.dma_start(yt[j - lag], u_tiles[j - lag][:])
            if variant == "dmaint":
                for j in range(L - lag, L):
                    nc.sync.dma_start(yt[j], u_tiles[j][:])

            if variant == "dmaint":
                pass
            elif variant == "dmain":
                nc.sync.dma_start(yt[0], u_tiles[0][:])
            elif variant == "dma":
                for j in range(L):
                    nc.sync.dma_start(yt[j], u_tiles[j][:])
            elif variant == "dmaout":
                for j in range(L):
                    nc.sync.dma_start(yt[j], u_tiles[0][:])
            elif variant == "outs":
                for j in range(0, L, 2):
                    for h in range(NCH):
                        ps_y = psypool.tile([128, CW], PS_DT, tag=f"y{h}",
                                            name=f"psy{j}_{h}")
                        nc.tensor.matmul(ps_y[:], wF(), u_tiles[j][:, cc(h)],
                                         start=True, stop=False)
                        nc.tensor.matmul(ps_y[:], w(W_E),
                                         u_tiles[j][:, cc(h)],
                                         start=False, stop=True)
                        y_sb = ypool.tile([128, CW], y_dt, tag="ysb",
                                          name=f"y{j}_{h}")
                        if (j + h) % 2 == 0:
                            nc.scalar.copy(y_sb[:], ps_y[:])
                        else:
                            nc.vector.tensor_copy(y_sb[:], ps_y[:])
                        nc.sync.dma_start(yt[j, :, cc(h)], y_sb[:])
                        ps_y2 = psypool.tile([128, CW], PS_DT, tag=f"y{h}",
                                             name=f"psy{j+1}_{h}")
                        nc.tensor.matmul(ps_y2[:], wF(),
                                         u_tiles[j + 1][:, cc(h)],
                                         start=True, stop=False)
                        nc.tensor.matmul(ps_y2[:], w(W_EB),
                                         u_tiles[j][:, cc(h)],
                                         start=False, stop=False)
                        nc.tensor.matmul(ps_y2[:], w(W_EA),
                                         u_tiles[j][:, cc(h)],
                                         start=False, stop=True)
                        y_sb2 = ypool.tile([128, CW], y_dt, tag="ysb",
                                           name=f"y{j+1}_{h}")
                        if (j + h) % 2 == 0:
                            nc.vector.tensor_copy(y_sb2[:], ps_y2[:])
                        else:
                            nc.scalar.copy(y_sb2[:], ps_y2[:])
                        nc.sync.dma_start(yt[j + 1, :, cc(h)], y_sb2[:])
            else:
                # ---- Phase A: double-step local scan; even-index slabs ----
                # w_tiles[i] = w_{2i} slab (i>=1); w_0 == 0.
                w_tiles = [None]
                for i in range(L // 2):
                    j = 2 * i            # consumes u_j, u_{j+1} -> w_{j+2}
                    w_n = xpool.tile([128, COLS], MM_DT, tag=f"w{i+1}",
                                     name=f"w{i+1}_{rep}")
                    for h in range(NCH):
                        ps = pspool.tile([128, CW], PS_DT, tag=f"scan{h}",
                                         name=f"ps{i}_{h}")
                        _mm(nc, ps, w(W_AB), u_tiles[j][:, cc(h)], CW,
                            start=True, stop=False)
                        if i > 0:
                            _mm(nc, ps, w(W_A2), w_tiles[i][:, cc(h)], CW,
                                start=False, stop=False)
                        _mm(nc, ps, wB(), u_tiles[j + 1][:, cc(h)], CW,
                            start=False, stop=True)
                        if h % 2 == 0:
                            nc.scalar.copy(w_n[:, cc(h)], ps[:])
                        else:
                            nc.vector.tensor_copy(w_n[:, cc(h)], ps[:])
                    w_tiles.append(w_n)

                if variant == "scan":
                    nc.sync.dma_start(yt[0], w_tiles[L // 2][:])
                else:
                    # ---- Phase B: chunk-start states (truncated conv) ----
                    # s_sb[:, 0:B_PER] is zeroed (chunk 0 starts from x=0),
                    # so all phase C matmuls can run full width.
                    r_tile = w_tiles[L // 2]
                    s_tiles = []
                    for h in range(NCH):
                        s_sb_h = spool.tile([128, CW], MM_DT, tag=f"s{h}",
                                            name=f"s_sb{h}_{rep}")
                        s_tiles.append(s_sb_h)
                    nc.gpsimd.memset(s_tiles[0][:, :B_PER], 0)
                    for h in range(NCH):
                        ps_s = pspool.tile([128, CW], PS_DT, tag=f"scan{h}",
                                           name=f"ps_s{h}")
                        lo = h * CW          # output col range [lo, hi)
                        for m in range(J):
                            sh = (m + 1) * B_PER
                            olo = max(lo, sh)
                            ncols = CW - (olo - lo)
                            _mm(nc, ps_s[:, olo - lo:CW], w(W_M0 + m),
                                r_tile[:, olo - sh:olo - sh + ncols], ncols,
                                start=(m == 0), stop=(m == J - 1))
                        olo = 0 if h > 0 else B_PER
                        if h % 2 == 0:
                            nc.scalar.copy(s_tiles[h][:, olo:CW],
                                           ps_s[:, olo:CW])
                        else:
                            nc.vector.tensor_copy(s_tiles[h][:, olo:CW],
                                                  ps_s[:, olo:CW])

                    # ---- Phase C: outputs, one pair (y_j, y_{j+1}) per
                    # corrected even state x_j = w_j + A^j s ----
                    #   y_j     = E x_j + F u_j
                    #   y_{j+1} = EA x_j + EB u_j + F u_{j+1}
                    # F u terms lead each group (no phase-B dep) so early
                    # groups can start under phase B.
                    for i in range(L // 2):
                        j = 2 * i
                        # corrected state x_j = w_j + A^j s (x_0 == s)
                        if i == 0:
                            x_h = [s_tiles[h][:] for h in range(NCH)]
                        else:
                            xc = xpool.tile([128, COLS], MM_DT, tag=f"xc{i}",
                                            name=f"xc{i}_{rep}")
                            for h in range(NCH):
                                ps_c = pspool.tile([128, CW], PS_DT,
                                                   tag=f"scan{h}",
                                                   name=f"ps_c{i}_{h}")
                                _mm(nc, ps_c, w(W_P0 + i - 1),
                                    s_tiles[h][:], CW, start=True, stop=True)
                                # x_j = A^j s + w_j (DVE; gpsimd can't read
                                # PSUM, and Act has no two-tensor add)
                                nc.vector.scalar_tensor_tensor(
                                    xc[:, cc(h)], ps_c[:], 1.0,
                                    w_tiles[i][:, cc(h)],
                                    op0=mybir.AluOpType.mult,
                                    op1=mybir.AluOpType.add)
                            x_h = [xc[:, cc(h)] for h in range(NCH)]

                        y_sb = ypool.tile([128, COLS], y_dt, tag="ysb",
                                          name=f"y{j}_{rep}")
                        y_sb2 = ypool.tile([128, COLS], y_dt, tag="ysb",
                                           name=f"y{j+1}_{rep}")
                        ps_ys, ps_ys2 = [], []
                        for h in range(NCH):
                            ps_y = psypool.tile([128, CW], PS_DT, tag=f"y{h}",
                                                name=f"psy{j}_{h}")
                            _mm(nc, ps_y, wF(), u_tiles[j][:, cc(h)], CW,
                                start=True, stop=False)
                            ps_ys.append(ps_y)
                            ps_y2 = psypool.tile([128, CW], PS_DT,
                                                 tag=f"y2{h}",
                                                 name=f"psy{j+1}_{h}")
                            _mm(nc, ps_y2, wF(), u_tiles[j + 1][:, cc(h)],
                                CW, start=True, stop=False)
                            _mm(nc, ps_y2, w(W_EB), u_tiles[j][:, cc(h)],
                                CW, start=False, stop=False)
                            ps_ys2.append(ps_y2)
                        for h in range(NCH):
                            xs = x_h[h]
                            _mm(nc, ps_ys[h], w(W_E), xs, CW,
                                start=False, stop=True)
                            _mm(nc, ps_ys2[h], w(W_EA), xs, CW,
                                start=False, stop=True)
                        # All y copies on Act: DVE is saturated by the
                        # corr-adds (gpsimd cannot touch PSUM).
                        ydma = nc.scalar if YDMA_ACT else nc.sync
                        for h in range(NCH):
                            nc.scalar.copy(y_sb[:, cc(h)], ps_ys[h][:])
                        ydma.dma_start(yt[j], y_sb[:])
                        for h in range(NCH):
                            nc.scalar.copy(y_sb2[:, cc(h)], ps_ys2[h][:])
                        ydma.dma_start(yt[j + 1], y_sb2[:])


_cached_nc = None


def _get_program():
    global _cached_nc
    if _cached_nc is None:
        _cached_nc = _build_program()
    return _cached_nc


def _make_weights(A, B, C, D):
    A = np.asarray(A, np.float64)
    Bm = np.asarray(B, np.float64)
    C = np.asarray(C, np.float64)
    Dm = np.asarray(D, np.float64)
    E = C @ A
    F = C @ Bm + Dm
    wts = np.zeros((NW, 128, 128), np.float64)
    wts[W_A2] = (A @ A).T
    wts[W_AB] = (A @ Bm).T
    wts[W_B] = Bm.T
    wts[W_E] = E.T
    wts[W_EA] = (E @ A).T
    wts[W_EB] = (E @ Bm).T
    wts[W_F] = F.T
    for i in range(1, L // 2):
        wts[W_P0 + i - 1] = np.linalg.matrix_power(A, 2 * i).T
    AL = np.linalg.matrix_power(A, L)
    Mm = np.eye(128)
    for m in range(J):
        wts[W_M0 + m] = Mm.T
        Mm = Mm @ AL
    # ship pre-transposed [128, NW*128] so the SBUF load is contiguous
    wts_t = wts.transpose(1, 0, 2).reshape(128, NW * 128)
    return np.ascontiguousarray(wts_t.astype(_np_dt()))


def make_in_maps(u, A, B, C, D):
    u = np.asarray(u, np.float32)
    wts = _make_weights(A, B, C, D)
    np_dt = _np_dt()
    in_maps = []
    for core in range(N_CORES):
        uc = u[core * B_PER:(core + 1) * B_PER]            # [4, T, 128]
        # ut[j, d, c*B_PER + b] = uc[b, c*L + j, d]
        ut = uc.reshape(B_PER, N1, L, DIM).transpose(2, 3, 1, 0)
        ut = np.ascontiguousarray(ut).reshape(L, 128, COLS).astype(np_dt)
        in_maps.append({"ut": ut, "wt": wts})
    return in_maps


def kernel(inputs, A, B, C, D):
    nc = _get_program()
    in_maps = make_in_maps(inputs, A, B, C, D)

    res = run_bass_kernel_spmd(nc, in_maps, core_ids=list(range(N_CORES)))

    out = np.empty((B_SZ, T, DIM), np.float32)
    for core in range(N_CORES):
        ytc = np.asarray(res.results[core]["yt"], np.float32)  # [L, 128, COLS]
        # out[b, c*L + j, d] = ytc[j, d, c*B_PER + b]
        oc = ytc.reshape(L, DIM, N1, B_PER).transpose(3, 2, 0, 1)
        out[core * B_PER:(core + 1) * B_PER] = oc.reshape(B_PER, T, DIM)
    return out



# revision 42
# speedup vs baseline: 1.0566x; 1.0566x over previous
"""Trainium2 Bass kernel for a linear state-space scan.

Reference computation (per batch row b):
    x_{t+1} = A x_t + B u_t          (x_0 = 0)
    out[t]  = C x_{t+1} + D u_t  =  E x_t + F u_t
with E = C A, F = C B + D.

Shapes: u [32, 4096, 128]; A, B, C, D [128, 128]; out [32, 4096, 128].

Strategy
--------
Data-parallel over batch: 32 rows / 8 cores = 4 rows per core; A/B/C/D-derived
weights replicated. No collectives.

Per core, time is split into N1 chunks of length L (L*N1 = 4096). Because
A = 0.9 * orthogonal, ||A^m|| = 0.9^m exactly, so chunk-start states are a
*short* truncated convolution over previous chunk contributions (J terms,
error ~0.9^(L*J)), making the whole computation chunk-parallel:

  1. Local scan (zero-init per chunk), all chunks in parallel as matmul
     columns:  w_{c,j+1} = A w_{c,j} + B u_{c,j}  -> L sequential matmuls
     over [128, N1*4] column slabs.  The slab is split into 512-column
     chunks forming independent scan chains, interleaved on PE so the
     PSUM->SBUF copy of one chain hides under the matmuls of others.
  2. Chunk-start states: s_c = sum_{m<J} (A^L)^m R_{c-1-m},  R_c = w_{c,L}.
  3. Outputs: out[c,j] = E w_{c,j} + F u_{c,j} + (E A^j) s_c.

Everything on-chip is kept transposed ([d=128 partitions, columns]); the
host does the (cheap) layout permutations during shard/unshard.
"""

import numpy as np

import concourse.tile as tile
from concourse import bacc, mybir
from concourse.bass_utils import run_bass_kernel_spmd

# Problem constants (hardcoded per contract)
B_SZ, T, DIM = 32, 4096, 128
N_CORES = 8
B_PER = B_SZ // N_CORES      # 4 batch rows per core

PS_DT = mybir.dt.float32

_MM_DTS = {"f32r": mybir.dt.float32r, "bf16": mybir.dt.bfloat16,
           "fp16": mybir.dt.float16}

# On-chip matmul dtype. "f32r" (fp32 storage, ~tf32 matmul), "bf16", or
# "fp16" (half storage, 10-bit mantissa -- 8x less quantization error than
# bf16 at the same speed; all values here are << fp16 range).
# NOTE: a bf16 matmul followed by an f32r matmul hangs the PE (weight-path
# hazard, repro.py) -- so 2-byte and f32r modes must not be mixed.
DT_MODE = "fp16"
MM_DT = _MM_DTS[DT_MODE]
Y_HALF = True                # ship y in the 2-byte dtype (halves DMA-out)


def set_dtypes(mode, y_half=None):
    global DT_MODE, MM_DT, Y_HALF
    DT_MODE = mode
    MM_DT = _MM_DTS[mode]
    if y_half is not None:
        Y_HALF = y_half


def _np_dt():
    import ml_dtypes
    return {"f32r": np.float32, "bf16": ml_dtypes.bfloat16,
            "fp16": np.float16}[DT_MODE]

CW = 512                     # matmul column-chunk width (one PSUM bank of f32)
PSY_BUFS = 1                 # 4 psum tags (y/y2 x chunk) -> 4 banks at bufs=1
SCAN_BUFS = 2
UNROLL = 4                   # bodies per hardware-loop iteration
WSPLIT = True                # split weight DMA into phased slices
YDMA_ACT = False             # y DMAs on SP (Act is busy with copies)
PSY_Y_BUFS = 1               # bufs for the early-closing y groups
CBAL = 1                     # phase-C copies moved from Act to DVE (0-2)


def set_chunk(Lnew, Jnew):
    """Set chunk length L (scan steps) and boundary truncation J.

    Double-step layout: the scan advances two steps at a time
    (w_{j+2} = A^2 w_j + AB u_j + B u_{j+1}), so only even-index state
    slabs exist. Weight slices:
      0:A^2  1:AB  2:B  3:E  4:EA  5:EB  6:F
      7..7+L/2-2:      A^j for even j in [2, L-2]
      7+L/2-1 .. +J-1: M_m = (A^L)^m truncated-conv weights
    """
    global L, N1, COLS, J, NW, NCH
    global W_A2, W_AB, W_B, W_E, W_EA, W_EB, W_F, W_P0, W_M0
    L = Lnew
    N1 = T // L
    COLS = N1 * B_PER
    J = Jnew
    W_A2, W_AB, W_B, W_E, W_EA, W_EB, W_F = 0, 1, 2, 3, 4, 5, 6
    W_P0 = 7                 # A^j, even j >= 2: index W_P0 + j//2 - 1
    W_M0 = W_P0 + L // 2 - 1
    NW = W_M0 + J
    NCH = COLS // CW


set_chunk(16, 5)             # 0.9^(16*5) ~ 2.2e-4 truncation, << fp16 noise



MM_SPLIT = 256               # moving-dim split: 256-col matmuls run at full
                             # PE clock on this part, 512-col ones ~1.5x slower


def _mm(nc, out_ap, w_ap, rhs_ap, ncols, start, stop):
    """matmul with the moving dim split into MM_SPLIT-col pieces."""
    lo = 0
    while lo < ncols:
        hi = min(lo + MM_SPLIT, ncols)
        nc.tensor.matmul(out_ap[:, lo:hi], w_ap, rhs_ap[:, lo:hi],
                         start=start and lo == 0,
                         stop=stop and hi == ncols)
        lo = hi


def _build_program(loop_reps=1, variant="full"):
    nc = bacc.Bacc("TRN2", target_bir_lowering=False, debug=False,
                   num_devices=N_CORES)
    u_dt = MM_DT
    y_dt = MM_DT if (Y_HALF and DT_MODE != "f32r") else PS_DT
    ut = nc.dram_tensor("ut", [L, 128, COLS], u_dt, kind="ExternalInput")
    wt = nc.dram_tensor("wt", [128, NW * 128], MM_DT, kind="ExternalInput")
    if variant in ("dma", "dmaint", "dmain", "dmaout"):
        yt_dt = u_dt
    elif variant == "scan":
        yt_dt = MM_DT
    else:
        yt_dt = y_dt
    yt = nc.dram_tensor("yt", [L, 128, COLS], yt_dt, kind="ExternalOutput")

    with tile.TileContext(nc) as tc:
        from contextlib import ExitStack
        ctx = ExitStack()
        with (
            # bufs=2 on cross-iteration tiles: under a hardware loop, a
            # single-buffered tile's WAR (next rep's DMA write vs this rep's
            # phase C read) blocks the SP queue head and serializes reps.
            tc.tile_pool(name="wts", bufs=2) as wpool,
            tc.tile_pool(name="u", bufs=2) as upool,
            tc.tile_pool(name="x", bufs=2) as xpool,
            tc.tile_pool(name="s", bufs=2) as spool,
            tc.tile_pool(name="y", bufs=6) as ypool,
            tc.tile_pool(name="ps", bufs=SCAN_BUFS, space="PSUM") as pspool,
            tc.tile_pool(name="psy", bufs=PSY_BUFS, space="PSUM") as psypool,
            ctx,
        ):
            wt_r = wt.ap().rearrange("p (n d) -> p n d", n=NW)

            def cc(h):  # column-chunk slice
                return slice(h * CW, (h + 1) * CW)

            unroll = 1
            if loop_reps > 1:
                # Unroll inside the hardware loop so double-buffered pools
                # actually alternate slots across consecutive reps.
                assert loop_reps % UNROLL == 0, "loop_reps % UNROLL != 0"
                ctx.enter_context(tc.For_i(0, loop_reps // UNROLL, 1))
                unroll = UNROLL

            for _rep in range(unroll):
                _emit_body(nc, tc, variant, _rep, wpool, upool, xpool, spool,
                           ypool, pspool, psypool, ut, wt_r, yt, u_dt, y_dt,
                           cc)

    nc.compile()
    return nc


def _emit_body(nc, tc, variant, rep, wpool, upool, xpool, spool, ypool,
               pspool, psypool, ut, wt_r, yt, u_dt, y_dt, cc):
    wtile = wpool.tile([128, NW, 128], MM_DT, tag="w", name=f"wtile{rep}")

    def w(i):
        return wtile[:, i, :]

    def wB():
        return w(W_B)

    def wF():
        return w(W_F)

    if True:
        if True:
            # Scan weights (A^2, AB, B) first (tiny DMA) so phase A can start
            # immediately; the remaining weight slices are interleaved into
            # the u stream early enough for phases B/C but without starving
            # phase A.
            wrest = [(W_E, W_P0), (W_P0, W_M0), (W_M0, NW)]
            wload_after = {6: 0, 10: 1, 12: 2}
            if WSPLIT:
                nc.sync.dma_start(wtile[:, :W_E], wt_r[:, :W_E])
            else:
                nc.sync.dma_start(wtile[:], wt_r[:])

            n_loads = 1 if variant == "dmaout" else L
            u_tiles = []
            lag = 4
            for j in range(n_loads):
                u_j = upool.tile([128, COLS], u_dt, tag=f"u{j}",
                                 name=f"u{j}_{rep}")
                nc.sync.dma_start(u_j[:], ut[j])
                u_tiles.append(u_j)
                if WSPLIT and j in wload_after:
                    lo, hi = wrest[wload_after[j]]
                    nc.sync.dma_start(wtile[:, lo:hi], wt_r[:, lo:hi])
                if variant == "dmaint" and j >= lag:
                    nc.sync.dma_start(yt[j - lag], u_tiles[j - lag][:])
            if variant == "dmaint":
                for j in range(L - lag, L):
                    nc.sync.dma_start(yt[j], u_tiles[j][:])

            if variant == "dmaint":
                pass
            elif variant == "dmain":
                nc.sync.dma_start(yt[0], u_tiles[0][:])
            elif variant == "dma":
                for j in range(L):
                    nc.sync.dma_start(yt[j], u_tiles[j][:])
            elif variant == "dmaout":
                for j in range(L):
                    nc.sync.dma_start(yt[j], u_tiles[0][:])
            elif variant == "outs":
                for j in range(0, L, 2):
                    for h in range(NCH):
                        ps_y = psypool.tile([128, CW], PS_DT, tag=f"y{h}",
                                            name=f"psy{j}_{h}")
                        nc.tensor.matmul(ps_y[:], wF(), u_tiles[j][:, cc(h)],
                                         start=True, stop=False)
                        nc.tensor.matmul(ps_y[:], w(W_E),
                                         u_tiles[j][:, cc(h)],
                                         start=False, stop=True)
                        y_sb = ypool.tile([128, CW], y_dt, tag="ysb",
                                          name=f"y{j}_{h}")
                        if (j + h) % 2 == 0:
                            nc.scalar.copy(y_sb[:], ps_y[:])
                        else:
                            nc.vector.tensor_copy(y_sb[:], ps_y[:])
                        nc.sync.dma_start(yt[j, :, cc(h)], y_sb[:])
                        ps_y2 = psypool.tile([128, CW], PS_DT, tag=f"y{h}",
                                             name=f"psy{j+1}_{h}")
                        nc.tensor.matmul(ps_y2[:], wF(),
                                         u_tiles[j + 1][:, cc(h)],
                                         start=True, stop=False)
                        nc.tensor.matmul(ps_y2[:], w(W_EB),
                                         u_tiles[j][:, cc(h)],
                                         start=False, stop=False)
                        nc.tensor.matmul(ps_y2[:], w(W_EA),
                                         u_tiles[j][:, cc(h)],
                                         start=False, stop=True)
                        y_sb2 = ypool.tile([128, CW], y_dt, tag="ysb",
                                           name=f"y{j+1}_{h}")
                        if (j + h) % 2 == 0:
                            nc.vector.tensor_copy(y_sb2[:], ps_y2[:])
                        else:
                            nc.scalar.copy(y_sb2[:], ps_y2[:])
                        nc.sync.dma_start(yt[j + 1, :, cc(h)], y_sb2[:])
            else:
                # ---- Phase A: double-step local scan; even-index slabs ----
                # w_tiles[i] = w_{2i} slab (i>=1); w_0 == 0.
                w_tiles = [None]
                for i in range(L // 2):
                    j = 2 * i            # consumes u_j, u_{j+1} -> w_{j+2}
                    w_n = xpool.tile([128, COLS], MM_DT, tag=f"w{i+1}",
                                     name=f"w{i+1}_{rep}")
                    for h in range(NCH):
                        ps = pspool.tile([128, CW], PS_DT, tag=f"scan{h}",
                                         name=f"ps{i}_{h}")
                        _mm(nc, ps, w(W_AB), u_tiles[j][:, cc(h)], CW,
                            start=True, stop=False)
                        if i > 0:
                            _mm(nc, ps, w(W_A2), w_tiles[i][:, cc(h)], CW,
                                start=False, stop=False)
                        _mm(nc, ps, wB(), u_tiles[j + 1][:, cc(h)], CW,
                            start=False, stop=True)
                        if h % 2 == 0:
                            nc.scalar.copy(w_n[:, cc(h)], ps[:])
                        else:
                            nc.vector.tensor_copy(w_n[:, cc(h)], ps[:])
                    w_tiles.append(w_n)

                if variant == "scan":
                    nc.sync.dma_start(yt[0], w_tiles[L // 2][:])
                else:
                    # ---- Phase B: chunk-start states (truncated conv) ----
                    # s_sb[:, 0:B_PER] is zeroed (chunk 0 starts from x=0),
                    # so all phase C matmuls can run full width.
                    r_tile = w_tiles[L // 2]
                    s_tiles = []
                    for h in range(NCH):
                        s_sb_h = spool.tile([128, CW], MM_DT, tag=f"s{h}",
                                            name=f"s_sb{h}_{rep}")
                        s_tiles.append(s_sb_h)
                    nc.gpsimd.memset(s_tiles[0][:, :B_PER], 0)
                    for h in range(NCH):
                        ps_s = pspool.tile([128, CW], PS_DT, tag=f"scan{h}",
                                           name=f"ps_s{h}")
                        lo = h * CW          # output col range [lo, hi)
                        for m in range(J):
                            sh = (m + 1) * B_PER
                            olo = max(lo, sh)
                            ncols = CW - (olo - lo)
                            _mm(nc, ps_s[:, olo - lo:CW], w(W_M0 + m),
                                r_tile[:, olo - sh:olo - sh + ncols], ncols,
                                start=(m == 0), stop=(m == J - 1))
                        olo = 0 if h > 0 else B_PER
                        if h % 2 == 0:
                            nc.scalar.copy(s_tiles[h][:, olo:CW],
                                           ps_s[:, olo:CW])
                        else:
                            nc.vector.tensor_copy(s_tiles[h][:, olo:CW],
                                                  ps_s[:, olo:CW])

                    # ---- Phase C: outputs, one pair (y_j, y_{j+1}) per
                    # corrected even state x_j = w_j + A^j s ----
                    #   y_j     = E x_j + F u_j
                    #   y_{j+1} = EA x_j + EB u_j + F u_{j+1}
                    # F u terms lead each group (no phase-B dep) so early
                    # groups can start under phase B.
                    for i in range(L // 2):
                        j = 2 * i
                        # corrected state x_j = w_j + A^j s (x_0 == s)
                        if i == 0:
                            x_h = [s_tiles[h][:] for h in range(NCH)]
                        else:
                            xc = xpool.tile([128, COLS], MM_DT, tag=f"xc{i}",
                                            name=f"xc{i}_{rep}")
                            for h in range(NCH):
                                ps_c = pspool.tile([128, CW], PS_DT,
                                                   tag=f"scan{h}",
                                                   name=f"ps_c{i}_{h}")
                                _mm(nc, ps_c, w(W_P0 + i - 1),
                                    s_tiles[h][:], CW, start=True, stop=True)
                                # x_j = A^j s + w_j (DVE; gpsimd can't read
                                # PSUM, and Act has no two-tensor add)
                                nc.vector.scalar_tensor_tensor(
                                    xc[:, cc(h)], ps_c[:], 1.0,
                                    w_tiles[i][:, cc(h)],
                                    op0=mybir.AluOpType.mult,
                                    op1=mybir.AluOpType.add)
                            x_h = [xc[:, cc(h)] for h in range(NCH)]

                        y_sb = ypool.tile([128, COLS], y_dt, tag="ysb",
                                          name=f"y{j}_{rep}")
                        y_sb2 = ypool.tile([128, COLS], y_dt, tag="ysb",
                                           name=f"y{j+1}_{rep}")
                        ps_ys, ps_ys2 = [], []
                        for h in range(NCH):
                            ps_y = psypool.tile([128, CW], PS_DT, tag=f"y{h}",
                                                bufs=PSY_Y_BUFS,
                                                name=f"psy{j}_{h}")
                            _mm(nc, ps_y, wF(), u_tiles[j][:, cc(h)], CW,
                                start=True, stop=False)
                            ps_ys.append(ps_y)
                            ps_y2 = psypool.tile([128, CW], PS_DT,
                                                 tag=f"y2{h}",
                                                 name=f"psy{j+1}_{h}")
                            _mm(nc, ps_y2, wF(), u_tiles[j + 1][:, cc(h)],
                                CW, start=True, stop=False)
                            _mm(nc, ps_y2, w(W_EB), u_tiles[j][:, cc(h)],
                                CW, start=False, stop=False)
                            ps_ys2.append(ps_y2)
                        for h in range(NCH):
                            xs = x_h[h]
                            _mm(nc, ps_ys[h], w(W_E), xs, CW,
                                start=False, stop=True)
                            _mm(nc, ps_ys2[h], w(W_EA), xs, CW,
                                start=False, stop=True)
                        # All y copies on Act: DVE is saturated by the
                        # corr-adds (gpsimd cannot touch PSUM).
                        ydma = nc.scalar if YDMA_ACT else nc.sync
                        for h in range(NCH):
                            if CBAL >= 1 and h == 1:
                                nc.vector.tensor_copy(y_sb[:, cc(h)],
                                                      ps_ys[h][:])
                            else:
                                nc.scalar.copy(y_sb[:, cc(h)], ps_ys[h][:])
                        ydma.dma_start(yt[j], y_sb[:])
                        for h in range(NCH):
                            if CBAL >= 2 and h == 1:
                                nc.vector.tensor_copy(y_sb2[:, cc(h)],
                                                      ps_ys2[h][:])
                            else:
                                nc.scalar.copy(y_sb2[:, cc(h)],
                                               ps_ys2[h][:])
                        ydma.dma_start(yt[j + 1], y_sb2[:])


_cached_nc = None


def _get_program():
    global _cached_nc
    if _cached_nc is None:
        _cached_nc = _build_program()
    return _cached_nc


def _make_weights(A, B, C, D):
    A = np.asarray(A, np.float64)
    Bm = np.asarray(B, np.float64)
    C = np.asarray(C, np.float64)
    Dm = np.asarray(D, np.float64)
    E = C @ A
    F = C @ Bm + Dm
    wts = np.zeros((NW, 128, 128), np.float64)
    wts[W_A2] = (A @ A).T
    wts[W_AB] = (A @ Bm).T
    wts[W_B] = Bm.T
    wts[W_E] = E.T
    wts[W_EA] = (E @ A).T
    wts[W_EB] = (E @ Bm).T
    wts[W_F] = F.T
    for i in range(1, L // 2):
        wts[W_P0 + i - 1] = np.linalg.matrix_power(A, 2 * i).T
    AL = np.linalg.matrix_power(A, L)
    Mm = np.eye(128)
    for m in range(J):
        wts[W_M0 + m] = Mm.T
        Mm = Mm @ AL
    # ship pre-transposed [128, NW*128] so the SBUF load is contiguous
    wts_t = wts.transpose(1, 0, 2).reshape(128, NW * 128)
    return np.ascontiguousarray(wts_t.astype(_np_dt()))


def make_in_maps(u, A, B, C, D):
    u = np.asarray(u, np.float32)
    wts = _make_weights(A, B, C, D)
    np_dt = _np_dt()
    in_maps = []
    for core in range(N_CORES):
        uc = u[core * B_PER:(core + 1) * B_PER]            # [4, T, 128]
        # ut[j, d, c*B_PER + b] = uc[b, c*L + j, d]
        ut = uc.reshape(B_PER, N1, L, DIM).transpose(2, 3, 1, 0)
        ut = np.ascontiguousarray(ut).reshape(L, 128, COLS).astype(np_dt)
        in_maps.append({"ut": ut, "wt": wts})
    return in_maps


def kernel(inputs, A, B, C, D):
    nc = _get_program()
    in_maps = make_in_maps(inputs, A, B, C, D)

    res = run_bass_kernel_spmd(nc, in_maps, core_ids=list(range(N_CORES)))

    out = np.empty((B_SZ, T, DIM), np.float32)
    for core in range(N_CORES):
        ytc = np.asarray(res.results[core]["yt"], np.float32)  # [L, 128, COLS]
        # out[b, c*L + j, d] = ytc[j, d, c*B_PER + b]
        oc = ytc.reshape(L, DIM, N1, B_PER).transpose(3, 2, 0, 1)
        out[core * B_PER:(core + 1) * B_PER] = oc.reshape(B_PER, T, DIM)
    return out

